# revision 44
# baseline (speedup 1.0000x reference)
"""Trainium2 Bass kernel for nn_ButterflyLayer2D (butterfly 2D CNN).

Strategy: pure data parallel over 8 NeuronCores (16 batch each).

Layouts (per core):
  - activations in SBUF as [128 = (w%2)*64 + c, (node, b, h, w//2)] so each
    2x2-stride-2 per-node conv is a q-parity scheme: K=128=(y,ci) in
    partitions, x (h-parity) accumulated over 2 matmuls, q (output w-parity)
    split over two col-tiled M=64 matmuls running concurrently on the PE.
    PSUM partitions then equal the destination layout, so every PSUM tile is
    evicted by ONE full-width contiguous relu+bias op (alternating
    ScalarE/VectorE).
  - input 4x4-patch conv: patches paired along y so K=32 (2 patches x 16)
    with block-diagonal weights, split into two col-tiled M=64 matmuls so
    they pair with level-1 matmuls; 4 batch-groups use row offsets
    0/32/64/96. Input tiles are interleaved 2:1 with the previous
    batch-half's level-1 tiles so the PE streams level-1 work while input
    evictions drain. A warmup matmul chain keeps the PE busy from ~5us so
    the HAM clock gate reaches 8/8 early and the stream never goes cold.
  - level 6 produces feats F2 [128=(node%2, c), (pair, b)]; the dense layer
    is row-tiled: even/odd nodes of a pair at row offsets 0/64 run
    concurrently with per-node [64,128] weights.
  - bulk weights (w2, w4, w6, wd) are prefetched on the GpSimd DMA queue,
    gated behind the input phase so they don't steal HBM bandwidth from the
    input blob; w5 reuses the input-blob SBUF slot once the input conv is
    done.
"""

import numpy as np
from contextlib import ExitStack

import concourse.bass as bass
import concourse.tile as tile
from concourse.tile import add_dep_helper
from concourse import bacc, mybir
from concourse.bass_utils import run_bass_kernel_spmd

F32 = mybir.dt.float32
BF16 = mybir.dt.bfloat16
AF = mybir.ActivationFunctionType
ALU = mybir.AluOpType

B, IN, NLVL, KLVL, C = 128, 256, 6, 3, 64
NK, OU, OV = 8, 8, 8
NCORES = 8
BC = B // NCORES          # 16 per-core batch
BG = BC
LVL_NODES = [4, 16, 64, 64, 64, 64]
LVL_HIN = [64, 32, 16, 8, 4, 2]
WGRP = 8                  # w3 streaming chunk (nodes)
BIAS_COLS = [4, 16, 64, 64, 64, 32]   # cols of each level's bias blob


# ----------------------------------------------------------------------------
# host-side pre-arrangement
# ----------------------------------------------------------------------------

def _prep_weights(inputs):
    """Weights/biases blobs shared by all cores."""
    import ml_dtypes
    out = {}
    # input filter: block-diagonal pair lhsT [32=(s,p,q), 128=(s,c)],
    # replicated at row bases 0/32/64/96 (4 concurrent batch groups)
    fin = inputs["in_filter"][:, :, 0, :].reshape(16, C).astype(np.float32)
    blk = np.zeros((32, 128), np.float32)
    blk[0:16, 0:64] = fin
    blk[16:32, 64:128] = fin
    finr = np.zeros((128, 128), np.float32)
    for g in range(4):
        finr[g * 32 : (g + 1) * 32] = blk
    out["fin"] = finr.astype(ml_dtypes.bfloat16)
    out["bin"] = np.concatenate([inputs["in_bias"], inputs["in_bias"]]).reshape(
        128, 1
    ).astype(np.float32)

    bias_blobs = []
    for lvl in range(1, NLVL + 1):
        f = inputs[f"f{lvl}"].astype(np.float32)  # [n,n,2,2,C,C] (x,y,ci,co)
        n = f.shape[0]
        assert n == 2 ** min(lvl, KLVL)
        # lhsT per node: [(y*64+ci), (x*64+co)] -> blob [128, n2*128]
        w = f.transpose(0, 1, 3, 4, 2, 5).reshape(n * n, 2 * C, 2 * C)
        out[f"w{lvl}"] = np.ascontiguousarray(w.transpose(1, 0, 2)).reshape(
            128, n * n * 128
        ).astype(ml_dtypes.bfloat16)
        b = inputs[f"b{lvl}"].astype(np.float32).reshape(n * n, C)
        if lvl < NLVL:
            bb = np.concatenate([b, b], axis=1)  # [nodes, 128] rows (q,c) dup
            bias_blobs.append(np.ascontiguousarray(bb.T))
        else:
            bb = b.reshape(n * n // 2, 2 * C)    # [pairs, (cA,cB)]
            bias_blobs.append(np.ascontiguousarray(bb.T))
    out["biases"] = np.ascontiguousarray(np.concatenate(bias_blobs, axis=1))
    # dense row-tiled: wd2 [128 = s*64 + c, pair*128 + (r*64 + ou*8 + ov)]
    wd = inputs["Wd"].astype(np.float32).reshape(NK * NK, 2, C, OU * OV)
    wd = wd.reshape(32, 2, 2, C, OU * OV)          # [pair, s, r, c, k2]
    wd = wd.transpose(1, 3, 0, 2, 4)               # [s, c, pair, r, k2]
    out["wd"] = np.ascontiguousarray(wd).reshape(128, 32 * 128).astype(
        ml_dtypes.bfloat16
    )
    return out


def _prep_input(in_data_core):
    """Per-core input blob [128, 8192]:
    row = (b%4)*32 + s*16 + p*4 + q ; col = (b//4)*2048 + x*32 + t
    value = in[b, 4x+p, 8t+4s+q]."""
    import ml_dtypes
    ind = in_data_core[:, :, :, 0]  # [16, 256, 256]
    a = ind.reshape(4, 4, 64, 4, 32, 2, 4)      # [half, g, x, p, t, s, q]
    a = a.transpose(1, 5, 3, 6, 0, 2, 4)        # [g, s, p, q, half, x, t]
    return np.ascontiguousarray(a).reshape(128, 8192).astype(ml_dtypes.bfloat16)


def _decode_output(t2_core):
    """t2 [128=(r,ou,ov), (s, pair, b)] -> [16, 64, 64, 2]."""
    t = t2_core.reshape(2, OU, OV, 2, 32, BG)   # r,ou,ov,s,p,b
    t = t.transpose(4, 3, 5, 0, 1, 2)           # p,s,b,r,ou,ov
    t = np.ascontiguousarray(t).reshape(8, 8, BG, 2, OU, OV)  # u,v,b,r,ou,ov
    t = t.transpose(2, 0, 4, 1, 5, 3)           # b,u,ou,v,ov,r
    return np.ascontiguousarray(t).reshape(BC, NK * OU, NK * OV, 2)


# ----------------------------------------------------------------------------
# device kernel
# ----------------------------------------------------------------------------

def _build_kernel(zero_bias=True):
    nc = bacc.Bacc(None, target_bir_lowering=False)
    p = {}
    p["a0"] = nc.declare_dram_parameter("a0", [128, 8192], BF16, isOutput=False)
    p["fin"] = nc.declare_dram_parameter("fin", [128, 128], BF16, isOutput=False)
    p["bin"] = nc.declare_dram_parameter("bin", [128, 1], F32, isOutput=False)
    for lvl in range(1, NLVL + 1):
        n2 = LVL_NODES[lvl - 1]
        p[f"w{lvl}"] = nc.declare_dram_parameter(f"w{lvl}", [128, n2 * 128], BF16, isOutput=False)
    p["biases"] = nc.declare_dram_parameter("biases", [128, sum(BIAS_COLS)], F32, isOutput=False)
    p["wd"] = nc.declare_dram_parameter("wd", [128, 32 * 128], BF16, isOutput=False)
    t2 = nc.declare_dram_parameter("t2", [128, 2 * 32 * BG], F32, isOutput=True)

    evict_ctr = [0]

    def evict(out_ap, psum_ap, bias_ap):
        """relu(psum + bias) -> sbuf, alternating engines to split the load.
        bias_ap None -> plain relu."""
        evict_ctr[0] += 1
        if evict_ctr[0] % 2 == 0:
            if bias_ap is None:
                return nc.scalar.activation(out_ap, psum_ap, AF.Relu)
            return nc.scalar.activation(out_ap, psum_ap, AF.Relu, bias=bias_ap)
        if bias_ap is None:
            return nc.vector.tensor_scalar_max(out_ap, psum_ap, 0.0)
        return nc.vector.tensor_scalar(out_ap, psum_ap, bias_ap, 0.0,
                                       op0=ALU.add, op1=ALU.max)

    with tile.TileContext(nc) as tc, ExitStack() as ctx:
        const = ctx.enter_context(tc.tile_pool(name="const", bufs=1))
        wpool = ctx.enter_context(tc.tile_pool(name="wts", bufs=3))
        apool = ctx.enter_context(tc.tile_pool(name="acts", bufs=1))
        inpool = ctx.enter_context(tc.tile_pool(name="inp", bufs=1))
        pfpool = ctx.enter_context(tc.tile_pool(name="pf", bufs=2))
        pdpool = ctx.enter_context(tc.tile_pool(name="pfd", bufs=1))
        fpool = ctx.enter_context(tc.tile_pool(name="feat", bufs=1))
        opool = ctx.enter_context(tc.tile_pool(name="outp", bufs=2))
        psA = ctx.enter_context(tc.tile_pool(name="psA", bufs=2, space="PSUM"))
        psB = ctx.enter_context(tc.tile_pool(name="psB", bufs=3, space="PSUM"))

        def ptile(name, cols=512):
            return psA.tile([128, cols], F32, tag="psA",
                            padded_shape=[128, 512], name=name)

        def ptileB(name):
            return psB.tile([128, 1024], F32, tag="psB",
                            padded_shape=[128, 1024], name=name)

        # ---------------- PE warmup ----------------
        # dummy matmuls on a zeroed tile keep the PE continuously busy
        # through the initial DMA window so the HAM clock gate reaches 8/8
        # before the real work starts and the stream never goes cold.
        warm = const.tile([128, 512], BF16, tag="warm", name="warm")
        nc.vector.memset(warm[:], 0.0)
        wtrash = const.tile([128, 8], F32, tag="wtrash", name="wtrash")
        # preload the ScalarE activation table during the startup dead zone
        nc.scalar.activation(wtrash[:], warm[:, :8], AF.Relu)
        wp = ptileB("warmps")
        for i in range(28):
            nc.tensor.matmul(wp[:, :512], warm[:, :128], warm[:],
                             start=True, stop=True)

        # ---------------- input + constants DMA (critical path order) -----
        a0s = inpool.tile([128, 8192], BF16, tag="a0w5", name="a0s")
        nc.sync.dma_start(a0s[:, 0:1024], p["a0"][:, 0:1024])
        w1t = wpool.tile([128, 1024], BF16, tag="wch", name="w1")
        nc.sync.dma_start(w1t[:, :512], p["w1"][:])
        fin_t = const.tile([128, 128], BF16)
        nc.sync.dma_start(fin_t[:], p["fin"][:])
        bin_t = const.tile([128, 1], F32)
        nc.sync.dma_start(bin_t[:], p["bin"][:])
        ball_t = const.tile([128, sum(BIAS_COLS)], F32, tag="biases", name="biases")
        boff = {}
        off = 0
        for lvl in range(1, NLVL + 1):
            boff[lvl] = off
            off += BIAS_COLS[lvl - 1]

        def bslice(lvl, n):
            return ball_t[:, boff[lvl] + n : boff[lvl] + n + 1]
        for h in range(1, 8):
            nc.sync.dma_start(a0s[:, h * 1024 : (h + 1) * 1024],
                              p["a0"][:, h * 1024 : (h + 1) * 1024])
        nc.sync.dma_start(ball_t[:], p["biases"][:])
        a0v = a0s[:].rearrange("p (h x t) -> p h x t", h=4, x=64)

        # ---------------- input conv + level 1 ----------------
        # X slab: [128=(y%2,c), (b, x=64, t=32)]
        X = apool.tile([128, BG * 64 * 32], BF16, tag="s0", name="x0")
        Xv = X[:].rearrange("p (b h w) -> p b h w", b=BG, h=64)
        A1 = apool.tile([128, 4 * BG * 32 * 16], BF16, tag="s1", name="a1")
        A1v = A1[:].rearrange("p (n b h w) -> p n b h w", n=4, b=BG, h=32)

        def in_tile(b, xh):
            # input conv, split into two col-tiled M=64 matmuls (the filter
            # is block-diagonal in (s, c)) so these pair with level 1's
            # col-tiled matmuls on the PE instead of serializing them
            g, half = b % 4, b // 4
            pt = ptile(f"pin{b}_{xh}")
            rhs = a0v[g * 32 : (g + 1) * 32, half,
                      xh * 16 : (xh + 1) * 16, :]
            for s in (0, 1):
                nc.tensor.matmul(
                    pt[s * 64 : (s + 1) * 64, :],
                    fin_t[g * 32 : (g + 1) * 32, s * 64 : (s + 1) * 64],
                    rhs,
                    start=True, stop=True,
                    tile_position=(g * 32, s * 64),
                )
            return evict(Xv[:, b, xh * 16 : (xh + 1) * 16, :], pt[:],
                         bin_t[:, 0:1])

        def l1_tile(n, b0):
            pt = ptileB(f"p1_{n}_{b0}")
            for i in range(2):
                bb = b0 + i
                for x in (0, 1):
                    for q in (0, 1):
                        rhs = Xv[:, bb, x::2, q::2]
                        nc.tensor.matmul(
                            pt[q * 64 : (q + 1) * 64,
                               i * 512 : (i + 1) * 512],
                            w1t[:, n * 128 + x * 64 :
                                n * 128 + (x + 1) * 64],
                            rhs,
                            start=(x == 0), stop=(x == 1),
                            skip_group_check=True,
                            tile_position=(0, q * 64),
                        )
            evict(A1v[:, n, b0 : b0 + 2, :, :], pt[:], bslice(1, n))

        # interleaved schedule: 16 input tiles per half, the previous half's
        # 8 level-1 tiles woven between them (so the PE streams level-1 work
        # while input evictions drain)
        gate = None
        for half in range(4):
            in_seq = [(half * 4 + g, xh) for xh in range(4) for g in range(4)]
            l1_seq = ([(n, (half - 1) * 4 + bp * 2) for n in range(4)
                       for bp in range(2)] if half > 0 else [])
            li = 0
            for idx, (b, xh) in enumerate(in_seq):
                gate = in_tile(b, xh)
                if idx % 2 == 1 and li < len(l1_seq):
                    l1_tile(*l1_seq[li])
                    li += 1
            while li < len(l1_seq):
                l1_tile(*l1_seq[li])
                li += 1

        # bulk weight prefetch on the gpsimd DMA queue, gated behind the
        # input phase so it doesn't steal HBM bandwidth from the input blob
        def gated_dma(dst, src):
            inst = nc.gpsimd.dma_start(dst, src)
            if inst is not None and gate is not None:
                add_dep_helper(inst.ins, gate.ins, reason="delay bulk weight prefetch")
            return inst

        w2c = []
        for h in range(2):
            w2t = wpool.tile([128, 1024], BF16, tag="wch", name=f"w2_{h}")
            gated_dma(w2t[:], p["w2"][:, h * 1024 : (h + 1) * 1024])
            w2c.append(w2t)
        w4t = pfpool.tile([128, 8192], BF16, tag="pf", name="w4")
        w6t = pfpool.tile([128, 8192], BF16, tag="pf", name="w6")
        for h in range(2):
            gated_dma(w4t[:, h * 4096 : (h + 1) * 4096],
                      p["w4"][:, h * 4096 : (h + 1) * 4096])
        for h in range(2):
            gated_dma(w6t[:, h * 4096 : (h + 1) * 4096],
                      p["w6"][:, h * 4096 : (h + 1) * 4096])
        wdt = pdpool.tile([128, 4096], BF16, tag="wd", name="wd")
        gated_dma(wdt[:], p["wd"][:])
        # w5 reuses the input-blob slot (waits for input conv to finish)
        w5t = inpool.tile([128, 8192], BF16, tag="a0w5", name="w5")
        for h in range(2):
            nc.gpsimd.dma_start(w5t[:, h * 4096 : (h + 1) * 4096],
                                p["w5"][:, h * 4096 : (h + 1) * 4096])

        # last half's level-1 tiles
        for n in range(4):
            for bp in range(2):
                l1_tile(n, 12 + bp * 2)

        # ---------------- level 2 ----------------
        A2 = apool.tile([128, 16 * BG * 16 * 8], BF16, tag="s0", name="a2")
        A2v = A2[:].rearrange("p (n b h w) -> p n b h w", n=16, b=BG, h=16)
        for t in range(2):
            for n in range(16):
                pn = (n // 4 // 2) * 2 + (n % 4) // 2
                nl = n % 8
                pt = ptileB(f"p2_{n}_{t}")
                for i in range(2):
                    bs = t * 8 + i * 4
                    for x in (0, 1):
                        for q in (0, 1):
                            rhs = A1v[:, pn, bs : bs + 4, x::2, q::2]
                            nc.tensor.matmul(
                                pt[q * 64 : (q + 1) * 64,
                                   i * 512 : (i + 1) * 512],
                                w2c[n // 8][:, nl * 128 + x * 64 :
                                            nl * 128 + (x + 1) * 64],
                                rhs,
                                start=(x == 0), stop=(x == 1),
                                skip_group_check=True,
                                tile_position=(0, q * 64),
                            )
                evict(A2v[:, n, t * 8 : (t + 1) * 8, :, :], pt[:],
                      bslice(2, n))

        # ---------------- level 3 (w3 streamed in chunks) ----------------
        A3 = apool.tile([128, 64 * BG * 8 * 4], BF16, tag="s1", name="a3")
        A3v = A3[:].rearrange("p (n b h w) -> p n b h w", n=64, b=BG, h=8)
        for g0 in range(0, 64, WGRP):
            w3t = wpool.tile([128, 1024], BF16, tag="wch", name=f"w3_{g0}")
            nc.sync.dma_start(w3t[:], p["w3"][:, g0 * 128 : (g0 + WGRP) * 128])
            if zero_bias:
                # adjacent even/odd nodes share a parent -> pair per tile
                for k in range(g0 // 2, (g0 + WGRP) // 2):
                    pn = (2 * k // 8 // 2) * 4 + (2 * k % 8) // 2
                    pt = ptileB(f"p3_{k}")
                    for j in (0, 1):
                        n = 2 * k + j
                        ln = n - g0
                        for x in (0, 1):
                            for q in (0, 1):
                                rhs = A2v[:, pn, :, x::2, q::2]
                                nc.tensor.matmul(
                                    pt[q * 64 : (q + 1) * 64,
                                       j * 512 : (j + 1) * 512],
                                    w3t[:, ln * 128 + x * 64 :
                                        ln * 128 + (x + 1) * 64],
                                    rhs,
                                    start=(x == 0), stop=(x == 1),
                                    skip_group_check=True,
                                    tile_position=(0, q * 64),
                                )
                    evict(A3v[:, 2 * k : 2 * k + 2, :, :, :], pt[:], None)
            else:
                for n in range(g0, g0 + WGRP):
                    ln = n - g0
                    pn = (n // 8 // 2) * 4 + (n % 8) // 2
                    pt = ptile(f"p3_{n}")
                    for x in (0, 1):
                        for q in (0, 1):
                            rhs = A2v[:, pn, :, x::2, q::2]
                            nc.tensor.matmul(
                                pt[q * 64 : (q + 1) * 64, :],
                                w3t[:, ln * 128 + x * 64 : ln * 128 + (x + 1) * 64],
                                rhs,
                                start=(x == 0), stop=(x == 1),
                                skip_group_check=True,
                                tile_position=(0, q * 64),
                            )
                    evict(A3v[:, n, :, :, :], pt[:], bslice(3, n))

        # ---------------- level 4 ----------------
        A4 = apool.tile([128, 64 * BG * 4 * 2], BF16, tag="s0", name="a4")
        A4v = A4[:].rearrange("p (n b h w) -> p n b h w", n=64, b=BG, h=4)
        if zero_bias:
            for grp in range(0, 64, 4):
                pt = ptile(f"p4_{grp}")
                for j in range(4):
                    n = grp + j
                    for x in (0, 1):
                        for q in (0, 1):
                            rhs = A3v[:, n, :, x::2, q::2]
                            nc.tensor.matmul(
                                pt[q * 64 : (q + 1) * 64,
                                   j * 128 : (j + 1) * 128],
                                w4t[:, n * 128 + x * 64 :
                                    n * 128 + (x + 1) * 64],
                                rhs,
                                start=(x == 0), stop=(x == 1),
                                skip_group_check=True,
                                tile_position=(0, q * 64),
                            )
                evict(A4v[:, grp : grp + 4, :, :, :], pt[:], None)
        else:
            for n in range(64):
                pt = ptile(f"p4_{n}")
                for x in (0, 1):
                    for q in (0, 1):
                        rhs = A3v[:, n, :, x::2, q::2]
                        nc.tensor.matmul(
                            pt[q * 64 : (q + 1) * 64, :128],
                            w4t[:, n * 128 + x * 64 : n * 128 + (x + 1) * 64],
                            rhs,
                            start=(x == 0), stop=(x == 1),
                            skip_group_check=True,
                            tile_position=(0, q * 64),
                        )
                evict(A4v[:, n, :, :, :], pt[:, :128], bslice(4, n))

        # ---------------- level 5 ----------------
        A5 = apool.tile([128, 64 * BG * 2 * 1], BF16, tag="s1", name="a5")
        A5v = A5[:].rearrange("p (n b h w) -> p n b h w", n=64, b=BG, h=2)
        if zero_bias:
            for grp in range(0, 64, 16):
                pt = ptile(f"p5_{grp}")
                for j in range(16):
                    n = grp + j
                    for x in (0, 1):
                        for q in (0, 1):
                            rhs = A4v[:, n, :, x::2, q::2]
                            nc.tensor.matmul(
                                pt[q * 64 : (q + 1) * 64,
                                   j * 32 : (j + 1) * 32],
                                w5t[:, n * 128 + x * 64 :
                                    n * 128 + (x + 1) * 64],
                                rhs,
                                start=(x == 0), stop=(x == 1),
                                skip_group_check=True,
                                tile_position=(0, q * 64),
                            )
                evict(A5v[:, grp : grp + 16, :, :, :], pt[:], None)
        else:
            for n in range(64):
                pt = ptile(f"p5_{n}")
                for x in (0, 1):
                    for q in (0, 1):
                        rhs = A4v[:, n, :, x::2, q::2]
                        nc.tensor.matmul(
                            pt[q * 64 : (q + 1) * 64, :32],
                            w5t[:, n * 128 + x * 64 : n * 128 + (x + 1) * 64],
                            rhs,
                            start=(x == 0), stop=(x == 1),
                            skip_group_check=True,
                            tile_position=(0, q * 64),
                        )
                evict(A5v[:, n, :, :, :], pt[:, :32], bslice(5, n))

        # ---------------- level 6: feats F2 [128=(s,c), (pair, b)] -------
        F2 = fpool.tile([128, 32 * BG], BF16, tag="feats", name="f2")
        F2v = F2[:].rearrange("p (n b) -> p n b", n=32)
        if zero_bias:
            for p0 in range(0, 32, 4):
                pt = ptile(f"p6_{p0}", 4 * BG)
                for j in range(4):
                    pr = p0 + j
                    for x in (0, 1):
                        for s in (0, 1):
                            node = 2 * pr + s
                            rhs = A5v[:, node, :, x, 0]
                            nc.tensor.matmul(
                                pt[s * 64 : (s + 1) * 64,
                                   j * BG : (j + 1) * BG],
                                w6t[:, node * 128 + x * 64 :
                                    node * 128 + (x + 1) * 64],
                                rhs,
                                start=(x == 0), stop=(x == 1),
                                skip_group_check=True,
                                tile_position=(0, s * 64),
                            )
                evict(F2v[:, p0 : p0 + 4, :], pt[:, : 4 * BG], None)
        else:
            for pr in range(32):
                pt = ptile(f"p6_{pr}", BG)
                for x in (0, 1):
                    for s in (0, 1):
                        node = 2 * pr + s
                        rhs = A5v[:, node, :, x, 0]
                        nc.tensor.matmul(
                            pt[s * 64 : (s + 1) * 64, :BG],
                            w6t[:, node * 128 + x * 64 :
                                node * 128 + (x + 1) * 64],
                            rhs,
                            start=(x == 0), stop=(x == 1),
                            skip_group_check=True,
                            tile_position=(0, s * 64),
                        )
                evict(F2v[:, pr, :], pt[:, :BG], bslice(6, pr))

        # ---------------- dense (row-tiled even/odd nodes) ----------------
        # t2s [128=(r,ou,ov), (s, pair, b)]
        t2s = {}
        for s in (0, 1):
            t2s[s] = opool.tile([128, 32 * BG], F32, tag="t2s", name=f"t2s{s}")
        for p0 in range(0, 32, 4):
            pts = {}
            for s in (0, 1):
                pts[s] = ptile(f"pd{s}_{p0}", 4 * BG)
            for j in range(4):
                pr = p0 + j
                for s in (0, 1):
                    nc.tensor.matmul(
                        pts[s][:, j * BG : (j + 1) * BG],
                        wdt[s * 64 : (s + 1) * 64, pr * 128 : (pr + 1) * 128],
                        F2v[s * 64 : (s + 1) * 64, pr, :],
                        start=True, stop=True,
                        tile_position=(s * 64, 0),
                    )
            for s in (0, 1):
                evict_ctr[0] += 1
                dst = t2s[s][:, p0 * BG : (p0 + 4) * BG]
                if evict_ctr[0] % 2 == 0:
                    nc.scalar.copy(dst, pts[s][:, : 4 * BG])
                else:
                    nc.vector.tensor_copy(dst, pts[s][:, : 4 * BG])
            for s in (0, 1):
                nc.sync.dma_start(
                    t2[:, s * 32 * BG + p0 * BG : s * 32 * BG + (p0 + 4) * BG],
                    t2s[s][:, p0 * BG : (p0 + 4) * BG],
                )
    nc.compile()
    return nc


# ----------------------------------------------------------------------------
# entry point
# ----------------------------------------------------------------------------

def _zero_bias(inputs):
    return all(
        float(np.abs(np.asarray(inputs[k])).max()) == 0.0
        for k in ("b3", "b4", "b5", "b6")
    )


def kernel(**inputs):
    inputs = {k: np.asarray(v) for k, v in inputs.items()}
    wblobs = _prep_weights(inputs)
    nc = _build_kernel(zero_bias=_zero_bias(inputs))
    in_maps = []
    for c in range(NCORES):
        m = dict(wblobs)
        m["a0"] = _prep_input(inputs["in_data"][c * BC : (c + 1) * BC])
        in_maps.append(m)
    res = run_bass_kernel_spmd(nc, in_maps, list(range(NCORES)))
    outs = [_decode_output(res.results[c]["t2"]) for c in range(NCORES)]
    return np.concatenate(outs, axis=0).astype(np.float32)


if __name__ == "__main__":
    import reference as ref

    inputs = {k: np.asarray(v) for k, v in ref.setup_inputs().items()}
    expected = np.asarray(ref.reference(**inputs))
    actual = kernel(**inputs)
    err = np.abs(actual - expected).max()
    rel = err / np.abs(expected).max()
    print("absmax:", err, "rel:", rel)
